# revision 6
# baseline (speedup 1.0000x reference)
"""Trainium2 Bass kernel for nn_DynamicPartitionMaskStitchModule.

The reference computes:
    order    = argsort(partitions, stable=True)   # a permutation of [0, N)
    gathered = data[order]
    out      = zeros_like(data).at[order].set(gathered)

Since `order` is a permutation, out[order[i]] = data[order[i]] for all i,
i.e. the stitch-scatter exactly inverts the partition-gather and the output
equals `data` bitwise. The memory-roofline implementation is therefore a
straight copy: each core reads its row shard of `data` from HBM and writes
it to the output buffer (read + write = the minimum possible HBM traffic
for this op). Rows are sharded N/8 per core; no cross-core communication.
"""

import sys

import numpy as np

for _p in ("/opt/trn_rl_repo", "/root/.axon_site/_ro/trn_rl_repo"):
    if _p not in sys.path:
        sys.path.append(_p)

from concourse import bass, mybir
from concourse import bass_utils
from concourse.bass_utils import run_bass_kernel_spmd


def _harden_tracing():
    """If the environment enables NTFF tracing (BASS_TRACE=1) but lacks the
    axon profile hook module or S3 artifact upload, degrade gracefully
    instead of crashing the run."""
    try:
        import antenv

        try:
            import antenv.axon_hooks  # noqa: F401
        except ImportError:
            import types

            mod = types.ModuleType("antenv.axon_hooks")
            state = {"hook": None}
            mod.set_axon_ntff_profile_hook = lambda h: state.__setitem__("hook", h)
            mod.get_axon_ntff_profile_hook = lambda: state["hook"]
            sys.modules["antenv.axon_hooks"] = mod
            antenv.axon_hooks = mod
            try:
                if "/root/.axon_site" not in sys.path:
                    sys.path.append("/root/.axon_site")
                from trn_agent_boot.trn_boot import _ntff_profile_via_ctypes

                hook = _ntff_profile_via_ctypes("/opt/axon/libaxon_pjrt.so")
                if hook is not None:
                    mod.set_axon_ntff_profile_hook(hook)
            except Exception:
                pass
    except Exception:
        pass

    orig_upload = bass_utils.upload_artifacts

    def _safe_upload(tmpdir):
        try:
            return orig_upload(tmpdir)
        except Exception:
            return f"local://{tmpdir}"

    bass_utils.upload_artifacts = _safe_upload


_harden_tracing()

N, D = 1_000_000, 128
N_CORES = 8
ROWS = N // N_CORES          # 125000 rows per core
ELEMS = ROWS * D             # 16M f32 = 64 MB per core
LANE = 250_000               # 1 MB lanes; ELEMS = 32 lane-pairs x 2 x LANE

_cached_nc = None


def _build():
    global _cached_nc
    if _cached_nc is not None:
        return _cached_nc

    # One large DMA per HWDGE ring (sync=SP and scalar=ACT), interleaved over
    # adjacent 1 MB lanes via the [32, 2, 250000] shape: sync copies [:,0,:]
    # (even lanes), scalar [:,1,:] (odd lanes). 1 MB is exactly the 16-engine
    # descriptor round-robin period (16 x 62.5 KB), so each SDMA engine's two
    # queue streams interleave into one near-sequential sweep of its stripe.
    # This beats both a contiguous half-split (tail-fragile under ambient HBM
    # load: two streams 32 MB apart) and finer lanes (fragment the engine
    # stripes). A single instruction per ring is critical: stacking several
    # instructions on one ring halves per-engine throughput.
    nc = bass.Bass()
    x = nc.declare_dram_parameter("x", [32, 2, LANE], mybir.dt.float32, isOutput=False)
    y = nc.declare_dram_parameter("y", [32, 2, LANE], mybir.dt.float32, isOutput=True)

    with nc.Block() as block, nc.semaphore("s0") as s0, nc.semaphore("s1") as s1:

        @block.sync
        def _(sync: bass.BassEngine):
            sync.dma_start(out=y[:, 0, :], in_=x[:, 0, :]).then_inc(s0, 16)
            sync.wait_ge(s0, 16)
            sync.wait_ge(s1, 16)

        @block.scalar
        def _(scalar: bass.BassEngine):
            scalar.dma_start(out=y[:, 1, :], in_=x[:, 1, :]).then_inc(s1, 16)

    _cached_nc = nc
    return nc


LAST_RESULTS = None  # BassKernelResults of the most recent run (for profiling)


def kernel(data: np.ndarray, partitions: np.ndarray = None, **_) -> np.ndarray:
    global LAST_RESULTS
    data = np.asarray(data)
    if data.dtype != np.float32 or not data.flags.c_contiguous:
        data = np.ascontiguousarray(data, dtype=np.float32)

    nc = _build()
    in_maps = [
        {"x": data[i * ROWS : (i + 1) * ROWS].reshape(32, 2, LANE)}
        for i in range(N_CORES)
    ]
    res = run_bass_kernel_spmd(nc, in_maps, core_ids=list(range(N_CORES)))
    LAST_RESULTS = res

    out = np.empty((N, D), dtype=np.float32)
    for i in range(N_CORES):
        out[i * ROWS : (i + 1) * ROWS] = np.asarray(res.results[i]["y"]).reshape(
            ROWS, D
        )
    return out
